# revision 26
# baseline (speedup 1.0000x reference)
"""Trainium2 Bass kernel for nn_CMAE_8856222564944 (retrieval_knn).

Computation (reference):
    h = L2-normalize rows of x            [B, N_ITEMS]
    h = tanh(h @ W1 + b1)                 [B, 600]
    h = tanh(h @ W2 + b2)                 [B, 200]
    h = tanh(h @ W3 + b3)                 [B, 600]
    dist = |h|^2 - 2 h @ E^T + |E|^2      [B, N_ITEMS]

Distribution (8 cores, tensor-parallel over the item dim):
    - x^T, W1, E^T are sharded over items (rows of W1/x^T, cols of E^T).
    - Each core computes a partial u^T = W1_sh^T x_sh^T; four pipelined
      AllReduces (one per 256-col B-chunk) of the small [600, 256] hidden;
      the W2/W3 layers are replicated.
    - Each core computes its column shard of dist and the host concatenates.

Schedule (v2):
    - Phase 1 is emitted as two pair-interleaved sweeps: B-chunk pairs
      (c0,c1) then (c2,c3) share one LDWEIGHTS per (k-tile, m-subtile)
      [the PE sequencer/LDWEIGHTS path is the phase-1 limiter, not FLOPs].
      PSUM banks 0-4 hold [128, 512] tiles whose halves are the two chunks.
    - x is loaded in column halves so chunk pair 0 completes (and its two
      AllReduces ring their doorbell) as early as possible; the collective
      chain (4 x ~15us serialized on the CC engine) is the critical path.
    - x/W1 stream over all four HWDGE queues (sync/pool for x halves,
      scalar/vector for W1), e8/et prefetch after, dist output writes
      alternate sync/pool.
    - dist runs k-OUTER sweeps over 6 PSUM banks: one LDWEIGHTS per 6
      448-col matmuls; PSUM evacuation alternates Vector/Scalar (a single
      engine cannot keep up with the PE).

Precision:
    - Big GEMMs in fp8-e4m3 DoubleRow; x pre-scaled by 128, W1 by 16 on
      host; 1/2048 folded into the tanh activation scale.
    - dist contraction: 2 fp8-DR k-tiles (h dims 0..511) + one bf16 tail
      (h dims 512..599 + hsq + ones aug rows), |E|^2 in the et tail.
    - MLP (W2/W3) bf16; fp32 PSUM accumulation everywhere; dist out bf16.
"""

import sys

if "/opt/trn_rl_repo" not in sys.path:
    sys.path.insert(0, "/opt/trn_rl_repo")

import numpy as np
import ml_dtypes

import concourse.bass as bass
import concourse.mybir as mybir
import concourse.tile as tile
from concourse import bacc

BF16 = ml_dtypes.bfloat16
FP8 = ml_dtypes.float8_e4m3
P = 128

# Full-size problem config
N_CORES = 8
B = 1024
H1 = 600
H2 = 200
N_ITEMS = 50000
ITEMS_PAD = 50176          # 8 * 6272, 6272 = 49 * 128
SH = ITEMS_PAD // N_CORES  # per-core item shard
H1P = 608                  # W1 free dim padded so the DoubleRow Ko step is 16B-aligned

X_SCALE = 128.0            # host pre-scale on normalized x before fp8 cast
W1_SCALE = 16.0            # host pre-scale on W1 before fp8 cast
U_SCALE = 1.0 / (X_SCALE * W1_SCALE)


def _chunks(total, size):
    """[(start, length), ...] covering [0, total) in `size` steps."""
    return [(s, min(size, total - s)) for s in range(0, total, size)]


def _dedup_ldweights(nc):
    """Post-scheduling: delete LDWEIGHTS whose stationary operand is identical
    to the previous LDWEIGHTS in the final PE stream (the array still holds
    those weights). Waits carried by a deleted load transfer to the next PE
    instruction so no dependency is lost."""
    import concourse.mybir as mb

    n_skipped = 0
    for bb in nc.main_func.blocks:
        insts = bb.instructions
        prev_key = None
        kill = {}
        for idx, ins in enumerate(insts):
            if getattr(ins, "engine", None) != mb.EngineType.PE:
                continue
            if isinstance(ins, mb.InstLdweights):
                key = (
                    str(ins.ins[0]),
                    ins.perf_mode,
                    ins.is_transpose,
                    ins.tile_position,
                    ins.tile_size,
                )
                if key == prev_key:
                    kill[idx] = ins
                else:
                    prev_key = key
            elif isinstance(ins, (mb.InstMatmult, mb.InstEventSemaphore, mb.InstNoOp)):
                pass  # these leave the loaded weights intact
            else:
                prev_key = None
        if not kill:
            continue
        new_insts = []
        pending = []
        for idx, ins in enumerate(insts):
            if idx in kill:
                pending.append(ins)
                continue
            if pending and getattr(ins, "engine", None) == mb.EngineType.PE:
                for dead in pending:
                    ins.merge_dependencies_from(dead)
                pending = []
            new_insts.append(ins)
        assert not pending
        bb.instructions = new_insts
        n_skipped += len(kill)
    print(f"_dedup_ldweights: removed {n_skipped} redundant weight loads")


def build_program(b=B, h1=H1, h2=H2, sh=SH, n_cores=N_CORES):
    """Build the per-core SPMD Bass program (same graph on every core)."""
    dt = mybir.dt
    fp32 = dt.float32
    bf16 = dt.bfloat16
    fp8 = dt.float8e4
    DR = mybir.MatmulPerfMode.DoubleRow

    assert sh % P == 0
    kdr = sh // 256                   # full DoubleRow item k-tiles (24)
    k_rem = sh - kdr * 256            # leftover rows (128) -> one normal k-tile
    assert k_rem in (0, P)
    nkt = kdr + 1
    mch = _chunks(h1, P)              # H1 row subtiles: [(0,128)..(512,88)]
    m2ch = _chunks(h2, P)             # H2 row subtiles: [(0,128),(128,72)]
    CL = 256                          # B-chunk width
    bch = _chunks(b, CL)              # 4 B-chunks
    nch = _chunks(sh, 448)            # dist output column tiles (14)
    n_dr = 2                          # DoubleRow k-tiles in dist (h dims 0..511)
    t_rows = h1 - n_dr * 256 + 2      # bf16 tail rows: h 512..599 + hsq + ones

    nc = bacc.Bacc(
        "TRN2",
        target_bir_lowering=False,
        debug=False,
        enable_asserts=False,
        num_devices=n_cores,
    )

    # x/W1 arrive host-packed in SBUF layout: [partition, k-tile, row-pair,
    # cols] so every DMA line is a multi-KB contiguous run.
    xT = nc.dram_tensor("xT", [P, 2, nkt, 2, 512], fp8, kind="ExternalInput")
    W1d = nc.dram_tensor("W1s", [P, nkt, 2, H1P], fp8, kind="ExternalInput")
    # W2/W3/biases host-packed partition-major: one DMA each (tiny 4-byte
    # descriptors for [ml,1] bias slices starved the scalar queue for ~35us).
    W2d = nc.dram_tensor("W2s", [P, len(_chunks(h1, P)), h2], bf16, kind="ExternalInput")
    W3d = nc.dram_tensor("W3s", [P, len(_chunks(h2, P)), h1], bf16, kind="ExternalInput")
    bd = nc.dram_tensor("bs", [P, 12], fp32, kind="ExternalInput")
    e8d = nc.dram_tensor("e8", [n_dr * 256, sh], fp8, kind="ExternalInput")
    etd = nc.dram_tensor("et", [t_rows, sh], bf16, kind="ExternalInput")
    outd = nc.dram_tensor("dist", [b, sh], bf16, kind="ExternalOutput")

    Tanh = mybir.ActivationFunctionType.Tanh
    Copy = mybir.ActivationFunctionType.Copy
    rg = [list(range(n_cores))]

    with tile.TileContext(nc) as tc:
        with (
            tc.tile_pool(name="persist", bufs=1) as persist,
            tc.tile_pool(name="dram", bufs=1, space="DRAM") as dram,
            tc.tile_pool(name="psum", bufs=1, space="PSUM") as psum_pool,
            tc.tile_pool(name="outs", bufs=8) as out_pool,
        ):
            # ---- persistent SBUF tensors -------------------------------
            # x^T in column halves, grouped 5 k-tiles per DMA (one tile per
            # (half, group): separate tiles stream in parallel on their
            # queues; big contiguous host-packed lines).
            KGS = [2, 5, 5, 5, 5, 3]          # k-tiles per DMA group
            assert sum(KGS) == nkt
            ngrp = len(KGS)
            KOFF = [sum(KGS[:i]) for i in range(ngrp)]
            T2G = [(g, t - KOFF[g]) for g in range(ngrp) for t in range(KOFF[g], KOFF[g] + KGS[g])]
            x_g = [
                [persist.tile([P, KGS[g], 2, 512], fp8, name=f"x_g{hf}_{g}")
                 for g in range(ngrp)]
                for hf in range(2)
            ]
            W1_g = [
                persist.tile([P, KGS[g], 2, H1P], fp8, name=f"W1_g{g}")
                for g in range(ngrp)
            ]
            e8_sb = persist.tile([P, 2 * n_dr, sh], fp8, name="e8_sb")
            et_sb = persist.tile([t_rows, sh], bf16, name="et_sb")
            W2_sb = persist.tile([P, len(mch), h2], bf16, name="W2_sb")
            W3_sb = persist.tile([P, len(m2ch), h1], bf16, name="W3_sb")
            # bias columns: b1 at [0:5], b2 at [5:7], b3 at [7:12]
            b_sb = persist.tile([P, 12], fp32, name="b_sb")
            ones_sb = persist.tile([P, len(mch), 1], bf16, name="ones_sb")
            up_sb = persist.tile([P, len(mch), b], bf16, name="up_sb")
            h1_sb = persist.tile([P, len(mch), b], bf16, name="h1_sb")
            h2_sb = persist.tile([P, len(m2ch), b], bf16, name="h2_sb")
            hh16_sb = persist.tile([P, len(mch), b], bf16, name="hh16_sb")
            hh8_sb = persist.tile([P, 2 * n_dr, b], fp8, name="hh8_sb")
            hq_sb = persist.tile([1, b], bf16, name="hq_sb")
            one_row_sb = persist.tile([1, b], bf16, name="one_row_sb")

            nc.vector.memset(ones_sb[:], 1.0)
            nc.vector.memset(one_row_sb[:], 1.0)

            # ---- dummy warm-up collective --------------------------------
            # The CC subsystem pays a one-time ~35-45us barrier plus a first-
            # op warmup; a tiny AllReduce with a doorbell at ~t=1us absorbs
            # both so the real chain starts hot.
            warm_b = dram.tile([1, P], bf16, name="warm_b")
            warm_r = dram.tile(
                [1, P], bf16,
                addr_space="Shared" if n_cores > 4 else "Local",
                name="warm_r",
            )
            nc.sync.dma_start(warm_b[0:1, :], one_row_sb[0:1, 0:P])
            nc.gpsimd.collective_compute(
                "AllReduce",
                mybir.AluOpType.add,
                replica_groups=rg,
                ins=[warm_b.opt()],
                outs=[warm_r.opt()],
            )

            # ---- bulk load emission --------------------------------------
            # Small tensors first on scalar (one packed DMA each).
            nc.scalar.dma_start(b_sb[:, :], bd[:, :])
            nc.scalar.dma_start(W2_sb[:, :, :], W2d[:, :, :])
            nc.scalar.dma_start(W3_sb[:, :, :], W3d[:, :, :])

            # W1 and x stream round-robin over the three DMA queues
            # (sync/pool/scalar), W1 + x-half0 first so chunk 0's k-loop
            # (and its AllReduce doorbell) completes earliest.
            QQ = [nc.sync, nc.gpsimd, nc.scalar]
            for g in range(ngrp):
                QQ[g % 3].dma_start(
                    W1_g[g][:, :, :, :], W1d[:, KOFF[g] : KOFF[g] + KGS[g], :, :]
                )
                QQ[(g + 1) % 3].dma_start(
                    x_g[0][g][:, :, :, :], xT[:, 0, KOFF[g] : KOFF[g] + KGS[g], :, :]
                )
            for g in range(ngrp):
                QQ[(g + 2) % 3].dma_start(
                    x_g[1][g][:, :, :, :], xT[:, 1, KOFF[g] : KOFF[g] + KGS[g], :, :]
                )

            def emit_eet_loads():
                # e8/et prefetch after x/W1 in queue order (needed only when
                # dist starts).
                for kk in range(2 * n_dr):
                    QQ[kk % 3].dma_start(
                        e8_sb[:, kk, :], e8d[kk * P : (kk + 1) * P, :]
                    )
                half = sh // 2
                nc.gpsimd.dma_start(et_sb[:, 0:half], etd[:, 0:half])
                nc.scalar.dma_start(et_sb[:, half:sh], etd[:, half:sh])
                # 'ones' aug row of the dist tail stationary (partition 89
                # is only reachable by DMA, not compute engines)
                nc.scalar.dma_start(
                    hh16_sb[89:90, len(mch) - 1, :], one_row_sb[0:1, :]
                )

            # ---- phase 1: partial u^T = W1_sh^T @ x_sh^T ----------------
            # Emitted as two pair-interleaved sweeps; each (k-tile,
            # m-subtile) stationary serves both chunks of the pair (the
            # second LDWEIGHTS is deduped post-scheduling).
            u_bounce = [
                dram.tile([h1, CL], bf16, name=f"u_bounce{ci}")
                for ci in range(len(bch))
            ]
            u_red = [
                dram.tile(
                    [h1, CL],
                    bf16,
                    addr_space="Shared" if n_cores > 4 else "Local",
                    name=f"u_red{ci}",
                )
                for ci in range(len(bch))
            ]

            # One PSUM bank hosts exactly ONE accumulation group at a time
            # (start=True clears the WHOLE bank). Chunks run sequentially;
            # per chunk, m-subtile mi accumulates in bank mi. The aliasing of
            # same-tag tiles serializes chunk c+1's first matmul after chunk
            # c's evacuation.
            for ci in range(len(bch)):
                hf, half_ci = divmod(ci, 2)
                cc = 256 * half_ci
                c0 = bch[ci][0]
                psA = [
                    psum_pool.tile(
                        [P, 256], fp32, name=f"p1_{ci}_{mi}", tag=f"pbank{mi}"
                    )
                    for mi in range(len(mch))
                ]
                for t in range(nkt):
                    last = t == kdr
                    g, ti = T2G[t]
                    for mi, (m0, ml) in enumerate(mch):
                        if not last:
                            nc.tensor.matmul(
                                psA[mi][:ml, :],
                                W1_g[g][:, ti, :, m0 : m0 + ml],
                                x_g[hf][g][:, ti, :, cc : cc + CL],
                                start=(t == 0),
                                stop=False,
                                perf_mode=DR,
                            )
                        else:
                            nc.tensor.matmul(
                                psA[mi][:ml, :],
                                W1_g[g][:, ti, 0, m0 : m0 + ml],
                                x_g[hf][g][:, ti, 0, cc : cc + CL],
                                start=False,
                                stop=True,
                            )
                # Evacuate (vector), bounce to DRAM (sync queue) and ring the
                # chunk's AllReduce doorbell.
                for mi, (m0, ml) in enumerate(mch):
                    nc.vector.tensor_copy(
                        up_sb[:ml, mi, c0 : c0 + CL], psA[mi][:ml, :]
                    )
                nc.sync.dma_start(
                    u_bounce[ci][0:512, :].rearrange("(k p) c -> p k c", k=4),
                    up_sb[:, 0:4, c0 : c0 + CL],
                )
                nc.sync.dma_start(
                    u_bounce[ci][512:h1, :], up_sb[:88, 4, c0 : c0 + CL]
                )
                nc.gpsimd.collective_compute(
                    "AllReduce",
                    mybir.AluOpType.add,
                    replica_groups=rg,
                    ins=[u_bounce[ci].opt()],
                    outs=[u_red[ci].opt()],
                )
                if ci == 1:
                    emit_eet_loads()

            # ---- per-B-chunk tail: tanh -> W2 -> W3 -> h_sq -> dist -----
            last_k = len(mch) - 1
            hrow = mch[-1][1]          # h_sq partition within last subtile (88)

            def emit_tanh(ci):
                c0, cl = bch[ci]
                nc.sync.dma_start(
                    up_sb[:, 0:4, c0 : c0 + cl],
                    u_red[ci][0:512, :].rearrange("(k p) c -> p k c", k=4),
                )
                nc.sync.dma_start(
                    up_sb[:88, 4, c0 : c0 + cl], u_red[ci][512:h1, :]
                )
                for mi, (m0, ml) in enumerate(mch):
                    nc.scalar.activation(
                        h1_sb[:ml, mi, c0 : c0 + cl],
                        up_sb[:ml, mi, c0 : c0 + cl],
                        Tanh,
                        bias=b_sb[:ml, mi : mi + 1],
                        scale=U_SCALE,
                    )

            def emit_mlp(ci):
                c0, cl = bch[ci]
                # phase 2 (banks 6/7 — one bank per accumulation group)
                for mi, (m0, ml) in enumerate(m2ch):
                    ps = psum_pool.tile(
                        [P, 256], fp32, name=f"p2_{ci}_{mi}", tag=f"pbank{6 + mi}"
                    )
                    for k, (r0, rl) in enumerate(mch):
                        nc.tensor.matmul(
                            ps[:ml, :cl],
                            W2_sb[:rl, k, m0 : m0 + ml],
                            h1_sb[:rl, k, c0 : c0 + cl],
                            start=(k == 0),
                            stop=(k == len(mch) - 1),
                        )
                    nc.scalar.activation(
                        h2_sb[:ml, mi, c0 : c0 + cl],
                        ps[:ml, :cl],
                        Tanh,
                        bias=b_sb[:ml, 5 + mi : 6 + mi],
                    )
                # phase 3 (alternates banks 6/7; tag aliasing serializes each
                # new tile after the previous same-bank tile's readers)
                for mi, (m0, ml) in enumerate(mch):
                    ps = psum_pool.tile(
                        [P, 256], fp32, name=f"p3_{ci}_{mi}",
                        tag=f"pbank{6 + mi % 2}",
                    )
                    for k, (r0, rl) in enumerate(m2ch):
                        nc.tensor.matmul(
                            ps[:ml, :cl],
                            W3_sb[:rl, k, m0 : m0 + ml],
                            h2_sb[:rl, k, c0 : c0 + cl],
                            start=(k == 0),
                            stop=(k == len(m2ch) - 1),
                        )
                    nc.scalar.activation(
                        hh16_sb[:ml, mi, c0 : c0 + cl],
                        ps[:ml, :cl],
                        Tanh,
                        bias=b_sb[:ml, 7 + mi : 8 + mi],
                    )
                    if mi < 2 * n_dr:  # fp8 copy for the DoubleRow dist k-tiles
                        nc.vector.tensor_copy(
                            hh8_sb[:ml, mi, c0 : c0 + cl],
                            hh16_sb[:ml, mi, c0 : c0 + cl],
                        )
                    # h^2 into dead h1_sb columns (input to the h_sq matmul)
                    nc.vector.tensor_tensor(
                        h1_sb[:ml, mi, c0 : c0 + cl],
                        hh16_sb[:ml, mi, c0 : c0 + cl],
                        hh16_sb[:ml, mi, c0 : c0 + cl],
                        mybir.AluOpType.mult,
                    )

            def emit_hsq(ci):
                c0, cl = bch[ci]
                psq = psum_pool.tile([1, 256], fp32, name=f"pq_{ci}", tag="pbank6")
                for k, (m0, ml) in enumerate(mch):
                    nc.tensor.matmul(
                        psq[:1, :cl],
                        ones_sb[:ml, k, 0:1],
                        h1_sb[:ml, k, c0 : c0 + cl],
                        start=(k == 0),
                        stop=(k == len(mch) - 1),
                    )
                nc.scalar.copy(hq_sb[0:1, c0 : c0 + cl], psq[:1, :cl])
                # h_sq aug row (partition 88 needs DMA, not compute engines)
                nc.scalar.dma_start(
                    hh16_sb[hrow : hrow + 1, last_k, c0 : c0 + cl],
                    hq_sb[0:1, c0 : c0 + cl],
                )

            # dist: per m-tile, k-outer sweeps over up to 6 psum banks.
            sweep_sizes = [6, 6, 2]        # 14 groups per m-tile
            assert sum(sweep_sizes) == len(nch)
            evac_eng = [0]                 # alternating toggle (persistent)
            out_q = [0]

            def emit_dist_mtile(mi, mid_hook=None):
                gi0 = 0
                for sw, swlen in enumerate(sweep_sizes):
                    grp = nch[gi0 : gi0 + swlen]
                    gi0 += swlen
                    pss = [
                        psum_pool.tile(
                            [P, 448], fp32, name=f"p4_{mi}_{sw}_{j}",
                            tag=f"pbank{j}",
                        )
                        for j in range(swlen)
                    ]
                    for k in range(n_dr):
                        for j, (n0, nl) in enumerate(grp):
                            nc.tensor.matmul(
                                pss[j][:P, :nl],
                                hh8_sb[:, 2 * k : 2 * k + 2, mi * P : (mi + 1) * P],
                                e8_sb[:, 2 * k : 2 * k + 2, n0 : n0 + nl],
                                start=(k == 0),
                                stop=False,
                                perf_mode=DR,
                            )
                    if mid_hook is not None:
                        # h_sq matmuls + aug-row DMA hide under the DR passes
                        mid_hook()
                        mid_hook = None
                    for j, (n0, nl) in enumerate(grp):
                        nc.tensor.matmul(
                            pss[j][:P, :nl],
                            hh16_sb[:t_rows, last_k, mi * P : (mi + 1) * P],
                            et_sb[:, n0 : n0 + nl],
                            start=False,
                            stop=True,
                        )
                    ot = None
                    for j, (n0, nl) in enumerate(grp):
                        if ot is None:
                            ot = out_pool.tile(
                                [P, 896], bf16, name=f"ot_{mi}_{sw}_{j}", tag="ot"
                            )
                            on0, off = n0, 0
                        if evac_eng[0] == 2:
                            nc.scalar.activation(
                                ot[:, off : off + nl], pss[j][:P, :nl], Copy
                            )
                        else:
                            nc.vector.tensor_copy(
                                ot[:, off : off + nl], pss[j][:P, :nl]
                            )
                        evac_eng[0] = (evac_eng[0] + 1) % 3
                        off += nl
                        if j == len(grp) - 1 or off == 896:
                            q = nc.sync if out_q[0] == 0 else nc.gpsimd
                            out_q[0] ^= 1
                            q.dma_start(
                                outd[mi * P : (mi + 1) * P, on0 : on0 + off],
                                ot[:, :off],
                            )
                            ot = None

            for ci in range(len(bch)):
                c0, cl = bch[ci]
                emit_tanh(ci)
                emit_mlp(ci)
                first = True
                for mi in range(c0 // P, (c0 + cl) // P):
                    emit_dist_mtile(
                        mi, mid_hook=(lambda c=ci: emit_hsq(c)) if first else None
                    )
                    first = False

    if not globals().get('NO_DEDUP'): _dedup_ldweights(nc)
    nc.compile()
    return nc


# ---------------------------------------------------------------------------
# Host side
# ---------------------------------------------------------------------------

def prep_inputs(x, W1, b1, W2, b2, W3, b3, item_emb, n_cores=N_CORES,
                items_pad=ITEMS_PAD):
    """Normalize/cast/transpose/pad/shard the full inputs -> per-core in_maps."""
    n_items = x.shape[1]
    b = x.shape[0]
    h1 = W1.shape[1]
    sh = items_pad // n_cores
    n_dr = 2
    t_rows = h1 - n_dr * 256 + 2

    x = np.asarray(x, np.float32)
    norm = np.sqrt((x * x).sum(axis=1, keepdims=True))
    xn = x / np.maximum(norm, 1e-12)

    xT = np.zeros((items_pad, b), dtype=FP8)
    xT[:n_items] = (xn.T * X_SCALE).astype(FP8)
    W1p = np.zeros((items_pad, H1P), dtype=FP8)
    W1p[:n_items, :h1] = (np.asarray(W1, np.float32) * W1_SCALE).astype(FP8)

    # Pack per-core x/W1 shards into the kernel's SBUF tile layout
    # [partition, (half,) k-tile, row-pair, cols] so each DMA line is a
    # multi-KB contiguous run. k-tiles padded to a multiple of 256 rows.
    nkt = (sh + 255) // 256
    rows_pad = nkt * 256

    def pack_x(shard):           # [sh, b] -> [P, 2, nkt, 2, 512]
        buf = np.zeros((rows_pad, b), dtype=FP8)
        buf[:sh] = shard
        # rows = t*256 + o*128 + p  ->  [t, o, p, half, c]
        v = buf.reshape(nkt, 2, 128, 2, 512)
        return np.ascontiguousarray(v.transpose(2, 3, 0, 1, 4))

    def pack_w1(shard):          # [sh, H1P] -> [P, nkt, 2, H1P]
        buf = np.zeros((rows_pad, H1P), dtype=FP8)
        buf[:sh] = shard
        v = buf.reshape(nkt, 2, 128, H1P)
        return np.ascontiguousarray(v.transpose(2, 0, 1, 3))

    E = np.asarray(item_emb, np.float32)
    m2eT = np.zeros((h1, items_pad), dtype=np.float32)
    m2eT[:, :n_items] = -2.0 * E.T
    e8 = m2eT[: n_dr * 256].astype(FP8)                 # h dims 0..511, fp8
    et = np.zeros((t_rows, items_pad), dtype=BF16)      # bf16 tail
    et[: h1 - n_dr * 256] = m2eT[n_dr * 256 :].astype(BF16)
    et[h1 - n_dr * 256, :] = np.ones((items_pad,), dtype=BF16)
    et[h1 - n_dr * 256 + 1, :n_items] = (E * E).sum(axis=1).astype(BF16)

    def pack_rows(w, dtype):
        """[rows, cols] -> [128, ceil(rows/128), cols] partition-major."""
        rows, cols = w.shape
        nk = (rows + 127) // 128
        buf = np.zeros((nk * 128, cols), dtype=dtype)
        buf[:rows] = np.asarray(w, np.float32).astype(dtype)
        return np.ascontiguousarray(buf.reshape(nk, 128, cols).transpose(1, 0, 2))

    bs = np.zeros((128, 12), dtype=np.float32)
    bs[:, 0:5] = pack_rows(np.asarray(b1, np.float32)[:, None], np.float32)[:, :, 0]
    bs[:, 5:7] = pack_rows(np.asarray(b2, np.float32)[:, None], np.float32)[:, :, 0]
    bs[:, 7:12] = pack_rows(np.asarray(b3, np.float32)[:, None], np.float32)[:, :, 0]
    common = {
        "W2s": pack_rows(W2, BF16),
        "W3s": pack_rows(W3, BF16),
        "bs": bs,
    }
    in_maps = []
    for c in range(n_cores):
        in_maps.append(
            dict(
                common,
                xT=pack_x(xT[c * sh : (c + 1) * sh]),
                W1s=pack_w1(W1p[c * sh : (c + 1) * sh]),
                e8=np.ascontiguousarray(e8[:, c * sh : (c + 1) * sh]),
                et=np.ascontiguousarray(et[:, c * sh : (c + 1) * sh]),
            )
        )
    return in_maps


_NC_CACHE = {}


def get_nc():
    if "nc" not in _NC_CACHE:
        _NC_CACHE["nc"] = build_program()
    return _NC_CACHE["nc"]


def kernel(x, W1, b1, W2, b2, W3, b3, item_emb, **run_kwargs):
    from concourse.bass_utils import run_bass_kernel_spmd

    n_items = x.shape[1]
    in_maps = prep_inputs(x, W1, b1, W2, b2, W3, b3, item_emb)
    nc = get_nc()
    res = run_bass_kernel_spmd(nc, in_maps, core_ids=list(range(N_CORES)), **run_kwargs)
    dist = np.concatenate(
        [res.results[c]["dist"] for c in range(N_CORES)], axis=1
    )[:, :n_items]
    if run_kwargs:
        kernel.last_results = res
    return np.ascontiguousarray(dist.astype(np.float32))


# revision 27
# speedup vs baseline: 1.0603x; 1.0603x over previous
"""Trainium2 Bass kernel for nn_CMAE_8856222564944 (retrieval_knn).

Computation (reference):
    h = L2-normalize rows of x            [B, N_ITEMS]
    h = tanh(h @ W1 + b1)                 [B, 600]
    h = tanh(h @ W2 + b2)                 [B, 200]
    h = tanh(h @ W3 + b3)                 [B, 600]
    dist = |h|^2 - 2 h @ E^T + |E|^2      [B, N_ITEMS]

Distribution (8 cores, tensor-parallel over the item dim):
    - x^T, W1, E^T are sharded over items (rows of W1/x^T, cols of E^T).
    - Each core computes a partial u^T = W1_sh^T x_sh^T; four pipelined
      AllReduces (one per 256-col B-chunk) of the small [600, 256] hidden;
      the W2/W3 layers are replicated.
    - Each core computes its column shard of dist and the host concatenates.

Schedule (v2):
    - Phase 1 is emitted as two pair-interleaved sweeps: B-chunk pairs
      (c0,c1) then (c2,c3) share one LDWEIGHTS per (k-tile, m-subtile)
      [the PE sequencer/LDWEIGHTS path is the phase-1 limiter, not FLOPs].
      PSUM banks 0-4 hold [128, 512] tiles whose halves are the two chunks.
    - x is loaded in column halves so chunk pair 0 completes (and its two
      AllReduces ring their doorbell) as early as possible; the collective
      chain (4 x ~15us serialized on the CC engine) is the critical path.
    - x/W1 stream over all four HWDGE queues (sync/pool for x halves,
      scalar/vector for W1), e8/et prefetch after, dist output writes
      alternate sync/pool.
    - dist runs k-OUTER sweeps over 6 PSUM banks: one LDWEIGHTS per 6
      448-col matmuls; PSUM evacuation alternates Vector/Scalar (a single
      engine cannot keep up with the PE).

Precision:
    - Big GEMMs in fp8-e4m3 DoubleRow; x pre-scaled by 128, W1 by 16 on
      host; 1/2048 folded into the tanh activation scale.
    - dist contraction: 2 fp8-DR k-tiles (h dims 0..511) + one bf16 tail
      (h dims 512..599 + hsq + ones aug rows), |E|^2 in the et tail.
    - MLP (W2/W3) bf16; fp32 PSUM accumulation everywhere; dist out bf16.
"""

import sys

if "/opt/trn_rl_repo" not in sys.path:
    sys.path.insert(0, "/opt/trn_rl_repo")

import numpy as np
import ml_dtypes

import concourse.bass as bass
import concourse.mybir as mybir
import concourse.tile as tile
from concourse import bacc

BF16 = ml_dtypes.bfloat16
FP8 = ml_dtypes.float8_e4m3
P = 128

# Full-size problem config
N_CORES = 8
B = 1024
H1 = 600
H2 = 200
N_ITEMS = 50000
ITEMS_PAD = 50176          # 8 * 6272, 6272 = 49 * 128
SH = ITEMS_PAD // N_CORES  # per-core item shard
H1P = 608                  # W1 free dim padded so the DoubleRow Ko step is 16B-aligned

X_SCALE = 128.0            # host pre-scale on normalized x before fp8 cast
W1_SCALE = 16.0            # host pre-scale on W1 before fp8 cast
U_SCALE = 1.0 / (X_SCALE * W1_SCALE)


def _chunks(total, size):
    """[(start, length), ...] covering [0, total) in `size` steps."""
    return [(s, min(size, total - s)) for s in range(0, total, size)]


def _dedup_ldweights(nc):
    """Post-scheduling: delete LDWEIGHTS whose stationary operand is identical
    to the previous LDWEIGHTS in the final PE stream (the array still holds
    those weights). Waits carried by a deleted load transfer to the next PE
    instruction so no dependency is lost."""
    import concourse.mybir as mb

    n_skipped = 0
    for bb in nc.main_func.blocks:
        insts = bb.instructions
        prev_key = None
        kill = {}
        for idx, ins in enumerate(insts):
            if getattr(ins, "engine", None) != mb.EngineType.PE:
                continue
            if isinstance(ins, mb.InstLdweights):
                key = (
                    str(ins.ins[0]),
                    ins.perf_mode,
                    ins.is_transpose,
                    ins.tile_position,
                    ins.tile_size,
                )
                if key == prev_key:
                    kill[idx] = ins
                else:
                    prev_key = key
            elif isinstance(ins, (mb.InstMatmult, mb.InstEventSemaphore, mb.InstNoOp)):
                pass  # these leave the loaded weights intact
            else:
                prev_key = None
        if not kill:
            continue
        new_insts = []
        pending = []
        for idx, ins in enumerate(insts):
            if idx in kill:
                pending.append(ins)
                continue
            if pending and getattr(ins, "engine", None) == mb.EngineType.PE:
                for dead in pending:
                    ins.merge_dependencies_from(dead)
                pending = []
            new_insts.append(ins)
        assert not pending
        bb.instructions = new_insts
        n_skipped += len(kill)
    print(f"_dedup_ldweights: removed {n_skipped} redundant weight loads")


def build_program(b=B, h1=H1, h2=H2, sh=SH, n_cores=N_CORES):
    """Build the per-core SPMD Bass program (same graph on every core)."""
    dt = mybir.dt
    fp32 = dt.float32
    bf16 = dt.bfloat16
    fp8 = dt.float8e4
    DR = mybir.MatmulPerfMode.DoubleRow

    assert sh % P == 0
    kdr = sh // 256                   # full DoubleRow item k-tiles (24)
    k_rem = sh - kdr * 256            # leftover rows (128) -> one normal k-tile
    assert k_rem in (0, P)
    nkt = kdr + 1
    mch = _chunks(h1, P)              # H1 row subtiles: [(0,128)..(512,88)]
    m2ch = _chunks(h2, P)             # H2 row subtiles: [(0,128),(128,72)]
    CL = 256                          # B-chunk width
    bch = _chunks(b, CL)              # 4 B-chunks
    nch = _chunks(sh, 448)            # dist output column tiles (14)
    n_dr = 2                          # DoubleRow k-tiles in dist (h dims 0..511)
    t_rows = h1 - n_dr * 256 + 2      # bf16 tail rows: h 512..599 + hsq + ones

    nc = bacc.Bacc(
        "TRN2",
        target_bir_lowering=False,
        debug=False,
        enable_asserts=False,
        num_devices=n_cores,
    )

    # x/W1 arrive host-packed in SBUF layout: [partition, k-tile, row-pair,
    # cols] so every DMA line is a multi-KB contiguous run.
    xT = nc.dram_tensor("xT", [P, 2, nkt, 2, 512], fp8, kind="ExternalInput")
    W1d = nc.dram_tensor("W1s", [P, nkt, 2, H1P], fp8, kind="ExternalInput")
    # W2/W3/biases host-packed partition-major: one DMA each (tiny 4-byte
    # descriptors for [ml,1] bias slices starved the scalar queue for ~35us).
    W2d = nc.dram_tensor("W2s", [P, len(_chunks(h1, P)), h2], bf16, kind="ExternalInput")
    W3d = nc.dram_tensor("W3s", [P, len(_chunks(h2, P)), h1], bf16, kind="ExternalInput")
    bd = nc.dram_tensor("bs", [P, 12], fp32, kind="ExternalInput")
    e8d = nc.dram_tensor("e8", [n_dr * 256, sh], fp8, kind="ExternalInput")
    etd = nc.dram_tensor("et", [t_rows, sh], bf16, kind="ExternalInput")
    outd = nc.dram_tensor("dist", [b, sh], bf16, kind="ExternalOutput")

    Tanh = mybir.ActivationFunctionType.Tanh
    Copy = mybir.ActivationFunctionType.Copy
    rg = [list(range(n_cores))]

    with tile.TileContext(nc) as tc:
        with (
            tc.tile_pool(name="persist", bufs=1) as persist,
            tc.tile_pool(name="dram", bufs=1, space="DRAM") as dram,
            tc.tile_pool(name="psum", bufs=1, space="PSUM") as psum_pool,
            tc.tile_pool(name="outs", bufs=8) as out_pool,
        ):
            # ---- persistent SBUF tensors -------------------------------
            # x^T in column halves, grouped 5 k-tiles per DMA (one tile per
            # (half, group): separate tiles stream in parallel on their
            # queues; big contiguous host-packed lines).
            KGS = [2, 5, 5, 5, 5, 3]          # k-tiles per DMA group
            assert sum(KGS) == nkt
            ngrp = len(KGS)
            KOFF = [sum(KGS[:i]) for i in range(ngrp)]
            T2G = [(g, t - KOFF[g]) for g in range(ngrp) for t in range(KOFF[g], KOFF[g] + KGS[g])]
            x_g = [
                [persist.tile([P, KGS[g], 2, 512], fp8, name=f"x_g{hf}_{g}")
                 for g in range(ngrp)]
                for hf in range(2)
            ]
            W1_g = [
                persist.tile([P, KGS[g], 2, H1P], fp8, name=f"W1_g{g}")
                for g in range(ngrp)
            ]
            e8_sb = persist.tile([P, 2 * n_dr, sh], fp8, name="e8_sb")
            et_sb = persist.tile([t_rows, sh], bf16, name="et_sb")
            W2_sb = persist.tile([P, len(mch), h2], bf16, name="W2_sb")
            W3_sb = persist.tile([P, len(m2ch), h1], bf16, name="W3_sb")
            # bias columns: b1 at [0:5], b2 at [5:7], b3 at [7:12]
            b_sb = persist.tile([P, 12], fp32, name="b_sb")
            ones_sb = persist.tile([P, len(mch), 1], bf16, name="ones_sb")
            up_sb = persist.tile([P, len(mch), b], bf16, name="up_sb")
            h1_sb = persist.tile([P, len(mch), b], bf16, name="h1_sb")
            h2_sb = persist.tile([P, len(m2ch), b], bf16, name="h2_sb")
            hh16_sb = persist.tile([P, len(mch), b], bf16, name="hh16_sb")
            hh8_sb = persist.tile([P, 2 * n_dr, b], fp8, name="hh8_sb")
            hq_sb = persist.tile([1, b], bf16, name="hq_sb")
            one_row_sb = persist.tile([1, b], bf16, name="one_row_sb")

            nc.vector.memset(ones_sb[:], 1.0)
            nc.vector.memset(one_row_sb[:], 1.0)

            # ---- dummy warm-up collective --------------------------------
            # The CC subsystem pays a one-time ~35-45us barrier plus a first-
            # op warmup; a tiny AllReduce with a doorbell at ~t=1us absorbs
            # both so the real chain starts hot.
            warm_b = dram.tile([1, P], bf16, name="warm_b")
            warm_r = dram.tile(
                [1, P], bf16,
                addr_space="Shared" if n_cores > 4 else "Local",
                name="warm_r",
            )
            nc.sync.dma_start(warm_b[0:1, :], one_row_sb[0:1, 0:P])
            nc.gpsimd.collective_compute(
                "AllReduce",
                mybir.AluOpType.add,
                replica_groups=rg,
                ins=[warm_b.opt()],
                outs=[warm_r.opt()],
            )

            # ---- bulk load emission --------------------------------------
            # Small tensors first on scalar (one packed DMA each).
            nc.scalar.dma_start(b_sb[:, :], bd[:, :])
            nc.scalar.dma_start(W2_sb[:, :, :], W2d[:, :, :])
            nc.scalar.dma_start(W3_sb[:, :, :], W3d[:, :, :])

            # W1 and x stream round-robin over the three DMA queues
            # (sync/pool/scalar), W1 + x-half0 first so chunk 0's k-loop
            # (and its AllReduce doorbell) completes earliest.
            QQ = [nc.sync, nc.gpsimd, nc.scalar]
            for g in range(ngrp):
                QQ[g % 3].dma_start(
                    W1_g[g][:, :, :, :], W1d[:, KOFF[g] : KOFF[g] + KGS[g], :, :]
                )
                QQ[(g + 1) % 3].dma_start(
                    x_g[0][g][:, :, :, :], xT[:, 0, KOFF[g] : KOFF[g] + KGS[g], :, :]
                )
            for g in range(ngrp):
                QQ[(g + 2) % 3].dma_start(
                    x_g[1][g][:, :, :, :], xT[:, 1, KOFF[g] : KOFF[g] + KGS[g], :, :]
                )

            def emit_eet_loads():
                # e8/et prefetch after x/W1 in queue order (needed only when
                # dist starts).
                for kk in range(2 * n_dr):
                    QQ[kk % 3].dma_start(
                        e8_sb[:, kk, :], e8d[kk * P : (kk + 1) * P, :]
                    )
                half = sh // 2
                nc.gpsimd.dma_start(et_sb[:, 0:half], etd[:, 0:half])
                nc.scalar.dma_start(et_sb[:, half:sh], etd[:, half:sh])
                # 'ones' aug row of the dist tail stationary (partition 89
                # is only reachable by DMA, not compute engines)
                nc.scalar.dma_start(
                    hh16_sb[89:90, len(mch) - 1, :], one_row_sb[0:1, :]
                )

            # ---- phase 1: partial u^T = W1_sh^T @ x_sh^T ----------------
            # Emitted as two pair-interleaved sweeps; each (k-tile,
            # m-subtile) stationary serves both chunks of the pair (the
            # second LDWEIGHTS is deduped post-scheduling).
            u_bounce = [
                dram.tile([h1, CL], bf16, name=f"u_bounce{ci}")
                for ci in range(len(bch))
            ]
            u_red = [
                dram.tile(
                    [h1, CL],
                    bf16,
                    addr_space="Shared" if n_cores > 4 else "Local",
                    name=f"u_red{ci}",
                )
                for ci in range(len(bch))
            ]

            # One PSUM bank hosts exactly ONE accumulation group at a time
            # (start=True clears the WHOLE bank). Chunks run sequentially;
            # per chunk, m-subtile mi accumulates in bank mi. The aliasing of
            # same-tag tiles serializes chunk c+1's first matmul after chunk
            # c's evacuation.
            for ci in range(len(bch)):
                hf, half_ci = divmod(ci, 2)
                cc = 256 * half_ci
                c0 = bch[ci][0]
                psA = [
                    psum_pool.tile(
                        [P, 256], fp32, name=f"p1_{ci}_{mi}", tag=f"pbank{mi}"
                    )
                    for mi in range(len(mch))
                ]
                for t in range(nkt):
                    last = t == kdr
                    g, ti = T2G[t]
                    for mi, (m0, ml) in enumerate(mch):
                        if not last:
                            nc.tensor.matmul(
                                psA[mi][:ml, :],
                                W1_g[g][:, ti, :, m0 : m0 + ml],
                                x_g[hf][g][:, ti, :, cc : cc + CL],
                                start=(t == 0),
                                stop=False,
                                perf_mode=DR,
                            )
                        else:
                            nc.tensor.matmul(
                                psA[mi][:ml, :],
                                W1_g[g][:, ti, 0, m0 : m0 + ml],
                                x_g[hf][g][:, ti, 0, cc : cc + CL],
                                start=False,
                                stop=True,
                            )
                # Evacuate (vector), bounce to DRAM (sync queue) and ring the
                # chunk's AllReduce doorbell.
                for mi, (m0, ml) in enumerate(mch):
                    nc.vector.tensor_copy(
                        up_sb[:ml, mi, c0 : c0 + CL], psA[mi][:ml, :]
                    )
                nc.sync.dma_start(
                    u_bounce[ci][0:512, :].rearrange("(k p) c -> p k c", k=4),
                    up_sb[:, 0:4, c0 : c0 + CL],
                )
                nc.sync.dma_start(
                    u_bounce[ci][512:h1, :], up_sb[:88, 4, c0 : c0 + CL]
                )
                nc.gpsimd.collective_compute(
                    "AllReduce",
                    mybir.AluOpType.add,
                    replica_groups=rg,
                    ins=[u_bounce[ci].opt()],
                    outs=[u_red[ci].opt()],
                )
                if ci == 2:
                    emit_eet_loads()

            # ---- per-B-chunk tail: tanh -> W2 -> W3 -> h_sq -> dist -----
            last_k = len(mch) - 1
            hrow = mch[-1][1]          # h_sq partition within last subtile (88)

            def emit_tanh(ci):
                c0, cl = bch[ci]
                nc.sync.dma_start(
                    up_sb[:, 0:4, c0 : c0 + cl],
                    u_red[ci][0:512, :].rearrange("(k p) c -> p k c", k=4),
                )
                nc.sync.dma_start(
                    up_sb[:88, 4, c0 : c0 + cl], u_red[ci][512:h1, :]
                )
                for mi, (m0, ml) in enumerate(mch):
                    nc.scalar.activation(
                        h1_sb[:ml, mi, c0 : c0 + cl],
                        up_sb[:ml, mi, c0 : c0 + cl],
                        Tanh,
                        bias=b_sb[:ml, mi : mi + 1],
                        scale=U_SCALE,
                    )

            def emit_mlp(ci):
                c0, cl = bch[ci]
                # phase 2 (banks 6/7 — one bank per accumulation group)
                for mi, (m0, ml) in enumerate(m2ch):
                    ps = psum_pool.tile(
                        [P, 256], fp32, name=f"p2_{ci}_{mi}", tag=f"pbank{6 + mi}"
                    )
                    for k, (r0, rl) in enumerate(mch):
                        nc.tensor.matmul(
                            ps[:ml, :cl],
                            W2_sb[:rl, k, m0 : m0 + ml],
                            h1_sb[:rl, k, c0 : c0 + cl],
                            start=(k == 0),
                            stop=(k == len(mch) - 1),
                        )
                    nc.scalar.activation(
                        h2_sb[:ml, mi, c0 : c0 + cl],
                        ps[:ml, :cl],
                        Tanh,
                        bias=b_sb[:ml, 5 + mi : 6 + mi],
                    )
                # phase 3 (alternates banks 6/7; tag aliasing serializes each
                # new tile after the previous same-bank tile's readers)
                for mi, (m0, ml) in enumerate(mch):
                    ps = psum_pool.tile(
                        [P, 256], fp32, name=f"p3_{ci}_{mi}",
                        tag=f"pbank{6 + mi % 2}",
                    )
                    for k, (r0, rl) in enumerate(m2ch):
                        nc.tensor.matmul(
                            ps[:ml, :cl],
                            W3_sb[:rl, k, m0 : m0 + ml],
                            h2_sb[:rl, k, c0 : c0 + cl],
                            start=(k == 0),
                            stop=(k == len(m2ch) - 1),
                        )
                    nc.scalar.activation(
                        hh16_sb[:ml, mi, c0 : c0 + cl],
                        ps[:ml, :cl],
                        Tanh,
                        bias=b_sb[:ml, 7 + mi : 8 + mi],
                    )
                    if mi < 2 * n_dr:  # fp8 copy for the DoubleRow dist k-tiles
                        nc.vector.tensor_copy(
                            hh8_sb[:ml, mi, c0 : c0 + cl],
                            hh16_sb[:ml, mi, c0 : c0 + cl],
                        )
                    # h^2 into dead h1_sb columns (input to the h_sq matmul)
                    nc.scalar.activation(
                        h1_sb[:ml, mi, c0 : c0 + cl],
                        hh16_sb[:ml, mi, c0 : c0 + cl],
                        mybir.ActivationFunctionType.Square,
                    )

            def emit_hsq(ci):
                c0, cl = bch[ci]
                psq = psum_pool.tile([1, 256], fp32, name=f"pq_{ci}", tag="pbank6")
                for k, (m0, ml) in enumerate(mch):
                    nc.tensor.matmul(
                        psq[:1, :cl],
                        ones_sb[:ml, k, 0:1],
                        h1_sb[:ml, k, c0 : c0 + cl],
                        start=(k == 0),
                        stop=(k == len(mch) - 1),
                    )
                nc.scalar.copy(hq_sb[0:1, c0 : c0 + cl], psq[:1, :cl])
                # h_sq aug row (partition 88 needs DMA, not compute engines)
                nc.scalar.dma_start(
                    hh16_sb[hrow : hrow + 1, last_k, c0 : c0 + cl],
                    hq_sb[0:1, c0 : c0 + cl],
                )

            # dist: per m-tile, k-outer sweeps over up to 6 psum banks.
            sweep_sizes = [6, 6, 2]        # 14 groups per m-tile
            assert sum(sweep_sizes) == len(nch)
            evac_eng = [0]                 # alternating toggle (persistent)
            out_q = [0]

            def emit_dist_mtile(mi, mid_hook=None):
                gi0 = 0
                for sw, swlen in enumerate(sweep_sizes):
                    grp = nch[gi0 : gi0 + swlen]
                    gi0 += swlen
                    pss = [
                        psum_pool.tile(
                            [P, 448], fp32, name=f"p4_{mi}_{sw}_{j}",
                            tag=f"pbank{j}",
                        )
                        for j in range(swlen)
                    ]
                    for k in range(n_dr):
                        for j, (n0, nl) in enumerate(grp):
                            nc.tensor.matmul(
                                pss[j][:P, :nl],
                                hh8_sb[:, 2 * k : 2 * k + 2, mi * P : (mi + 1) * P],
                                e8_sb[:, 2 * k : 2 * k + 2, n0 : n0 + nl],
                                start=(k == 0),
                                stop=False,
                                perf_mode=DR,
                            )
                    if mid_hook is not None:
                        # h_sq matmuls + aug-row DMA hide under the DR passes
                        mid_hook()
                        mid_hook = None
                    for j, (n0, nl) in enumerate(grp):
                        nc.tensor.matmul(
                            pss[j][:P, :nl],
                            hh16_sb[:t_rows, last_k, mi * P : (mi + 1) * P],
                            et_sb[:, n0 : n0 + nl],
                            start=False,
                            stop=True,
                        )
                    ot = None
                    for j, (n0, nl) in enumerate(grp):
                        if ot is None:
                            ot = out_pool.tile(
                                [P, 896], bf16, name=f"ot_{mi}_{sw}_{j}", tag="ot"
                            )
                            on0, off = n0, 0
                            eng = evac_eng[0]
                            evac_eng[0] ^= 1
                        if eng == 0:
                            nc.vector.tensor_copy(
                                ot[:, off : off + nl], pss[j][:P, :nl]
                            )
                        else:
                            nc.scalar.activation(
                                ot[:, off : off + nl], pss[j][:P, :nl], Copy
                            )
                        off += nl
                        if j == len(grp) - 1 or off == 896:
                            q = nc.sync if out_q[0] == 0 else nc.gpsimd
                            out_q[0] ^= 1
                            q.dma_start(
                                outd[mi * P : (mi + 1) * P, on0 : on0 + off],
                                ot[:, :off],
                            )
                            ot = None

            for ci in range(len(bch)):
                c0, cl = bch[ci]
                emit_tanh(ci)
                emit_mlp(ci)
                first = True
                for mi in range(c0 // P, (c0 + cl) // P):
                    emit_dist_mtile(
                        mi, mid_hook=(lambda c=ci: emit_hsq(c)) if first else None
                    )
                    first = False

    if not globals().get('NO_DEDUP'): _dedup_ldweights(nc)
    nc.compile()
    return nc


# ---------------------------------------------------------------------------
# Host side
# ---------------------------------------------------------------------------

def prep_inputs(x, W1, b1, W2, b2, W3, b3, item_emb, n_cores=N_CORES,
                items_pad=ITEMS_PAD):
    """Normalize/cast/transpose/pad/shard the full inputs -> per-core in_maps."""
    n_items = x.shape[1]
    b = x.shape[0]
    h1 = W1.shape[1]
    sh = items_pad // n_cores
    n_dr = 2
    t_rows = h1 - n_dr * 256 + 2

    x = np.asarray(x, np.float32)
    norm = np.sqrt((x * x).sum(axis=1, keepdims=True))
    xn = x / np.maximum(norm, 1e-12)

    xT = np.zeros((items_pad, b), dtype=FP8)
    xT[:n_items] = (xn.T * X_SCALE).astype(FP8)
    W1p = np.zeros((items_pad, H1P), dtype=FP8)
    W1p[:n_items, :h1] = (np.asarray(W1, np.float32) * W1_SCALE).astype(FP8)

    # Pack per-core x/W1 shards into the kernel's SBUF tile layout
    # [partition, (half,) k-tile, row-pair, cols] so each DMA line is a
    # multi-KB contiguous run. k-tiles padded to a multiple of 256 rows.
    nkt = (sh + 255) // 256
    rows_pad = nkt * 256

    def pack_x(shard):           # [sh, b] -> [P, 2, nkt, 2, 512]
        buf = np.zeros((rows_pad, b), dtype=FP8)
        buf[:sh] = shard
        # rows = t*256 + o*128 + p  ->  [t, o, p, half, c]
        v = buf.reshape(nkt, 2, 128, 2, 512)
        return np.ascontiguousarray(v.transpose(2, 3, 0, 1, 4))

    def pack_w1(shard):          # [sh, H1P] -> [P, nkt, 2, H1P]
        buf = np.zeros((rows_pad, H1P), dtype=FP8)
        buf[:sh] = shard
        v = buf.reshape(nkt, 2, 128, H1P)
        return np.ascontiguousarray(v.transpose(2, 0, 1, 3))

    E = np.asarray(item_emb, np.float32)
    m2eT = np.zeros((h1, items_pad), dtype=np.float32)
    m2eT[:, :n_items] = -2.0 * E.T
    e8 = m2eT[: n_dr * 256].astype(FP8)                 # h dims 0..511, fp8
    et = np.zeros((t_rows, items_pad), dtype=BF16)      # bf16 tail
    et[: h1 - n_dr * 256] = m2eT[n_dr * 256 :].astype(BF16)
    et[h1 - n_dr * 256, :] = np.ones((items_pad,), dtype=BF16)
    et[h1 - n_dr * 256 + 1, :n_items] = (E * E).sum(axis=1).astype(BF16)

    def pack_rows(w, dtype):
        """[rows, cols] -> [128, ceil(rows/128), cols] partition-major."""
        rows, cols = w.shape
        nk = (rows + 127) // 128
        buf = np.zeros((nk * 128, cols), dtype=dtype)
        buf[:rows] = np.asarray(w, np.float32).astype(dtype)
        return np.ascontiguousarray(buf.reshape(nk, 128, cols).transpose(1, 0, 2))

    bs = np.zeros((128, 12), dtype=np.float32)
    bs[:, 0:5] = pack_rows(np.asarray(b1, np.float32)[:, None], np.float32)[:, :, 0]
    bs[:, 5:7] = pack_rows(np.asarray(b2, np.float32)[:, None], np.float32)[:, :, 0]
    bs[:, 7:12] = pack_rows(np.asarray(b3, np.float32)[:, None], np.float32)[:, :, 0]
    common = {
        "W2s": pack_rows(W2, BF16),
        "W3s": pack_rows(W3, BF16),
        "bs": bs,
    }
    in_maps = []
    for c in range(n_cores):
        in_maps.append(
            dict(
                common,
                xT=pack_x(xT[c * sh : (c + 1) * sh]),
                W1s=pack_w1(W1p[c * sh : (c + 1) * sh]),
                e8=np.ascontiguousarray(e8[:, c * sh : (c + 1) * sh]),
                et=np.ascontiguousarray(et[:, c * sh : (c + 1) * sh]),
            )
        )
    return in_maps


_NC_CACHE = {}


def get_nc():
    if "nc" not in _NC_CACHE:
        _NC_CACHE["nc"] = build_program()
    return _NC_CACHE["nc"]


def kernel(x, W1, b1, W2, b2, W3, b3, item_emb, **run_kwargs):
    from concourse.bass_utils import run_bass_kernel_spmd

    n_items = x.shape[1]
    in_maps = prep_inputs(x, W1, b1, W2, b2, W3, b3, item_emb)
    nc = get_nc()
    res = run_bass_kernel_spmd(nc, in_maps, core_ids=list(range(N_CORES)), **run_kwargs)
    dist = np.concatenate(
        [res.results[c]["dist"] for c in range(N_CORES)], axis=1
    )[:, :n_items]
    if run_kwargs:
        kernel.last_results = res
    return np.ascontiguousarray(dist.astype(np.float32))
